# revision 11
# baseline (speedup 1.0000x reference)
"""Single-head causal attention (B=4, T=2048, C=2048, H=128) on 8 TRN2 cores.

Sharding: 2 cores per batch (b = core//2, par = core%2), flash-style split
over KEYS. Core (b, par) owns the 8 interleaved key tiles {2m+par}; it
computes K^T/V^T for those 1024 keys only and Q^T for ALL 2048 queries, then
accumulates partial attention sums (O^T = sum_k exp(s) v, den = sum_k exp(s))
over its own keys for every query. The host combines sibling partials:
O = (ot0+ot1)/(den0+den1). Softmax max-shift is skipped (|s| < ~6 here), so
partials combine exactly.

The host permutes x[b]^T columns to [own tiles || sibling tiles], shipped as
two tensors; the own half loads FIRST (all x on one DMA ring, strictly
ordered, so the halves do not compete for HBM bandwidth). K/V/Q-own compute
from the own half; pass-1 attention (own-q columns) overlaps the sibling-half
DMA and the Q-sib projection, which are interleaved into the pass-1 stream.

Causality per own key tile m (global tile 2m+par):
  own-q cols:  window [128m, 1024); first block is the diagonal -> tri mask.
  sib-q cols:  window [128m, 1024)+1024; first block fully allowed (par=0)
    or fully masked (par=1) -> multiply by nodd = 1-par.

PSUM is 4 pair-tags (4KB each); attention carves independent accumulation
regions out of pair tiles by slicing (den vectors live in spare partition
rows, 32-aligned for col_grp). Attention is software-pipelined (S matmuls
skewed ahead of O/den to hide the PE->ACT(exp)->DVE(mask)->PE round-trip);
pass 2 uses full-window segments with one big exp per key tile (ACT
instruction overhead is the attention floor). ~26 dummy warmup matmuls hold
the PE HAM clock warm while the first x chunk loads.
"""

import numpy as np
import ml_dtypes

B, T, C, H = 4, 2048, 2048, 128
P = 128                 # tile edge
NCT = C // P            # 16 contraction chunks
NOT = 8                 # owned key tiles per core
NQ = NOT * P            # 1024 owned key rows per core
N_CORES = 8
SCALE = float(H) ** -0.5
BF16 = ml_dtypes.bfloat16
WIDE = False            # single 1024-col matmuls (ISA bf16 moving max)

_cache = {}


def _build():
    import concourse.bass as bass
    import concourse.mybir as mybir
    import concourse.tile as tile
    from concourse import bacc
    from concourse.masks import make_identity, make_upper_triangular

    dt = mybir.dt
    nc = bacc.Bacc(
        "TRN2",
        target_bir_lowering=False,
        debug=False,
        enable_asserts=False,
        num_devices=N_CORES,
    )

    xo_d = nc.dram_tensor("xo", [C, NQ], dt.bfloat16, kind="ExternalInput").ap()
    xs_d = nc.dram_tensor("xs", [C, NQ], dt.bfloat16, kind="ExternalInput").ap()
    wq_d = nc.dram_tensor("wq", [P, NCT, H], dt.bfloat16, kind="ExternalInput").ap()
    wk_d = nc.dram_tensor("wk", [P, NCT, H], dt.bfloat16, kind="ExternalInput").ap()
    wv_d = nc.dram_tensor("wv", [P, NCT, H], dt.bfloat16, kind="ExternalInput").ap()
    nodd_d = nc.dram_tensor("nodd", [P, 1], dt.float32, kind="ExternalInput").ap()
    ot_d = nc.dram_tensor("ot", [H, T], dt.float32, kind="ExternalOutput").ap()
    den_d = nc.dram_tensor("den", [1, T], dt.float32, kind="ExternalOutput").ap()

    XJ = 2          # c-tiles per x chunk
    NG = NCT // XJ  # 8 chunks per x half

    with tile.TileContext(nc) as tc:
        with (
            tc.tile_pool(name="persist", bufs=1) as persist,
            tc.tile_pool(name="ephem", bufs=6) as ephem,
            tc.tile_pool(name="outp", bufs=2) as outp,
            tc.tile_pool(name="psum", bufs=1, space="PSUM") as psum,
        ):
            def pair(t, shape=(P, 2 * 512), dtype=dt.float32, name="pp"):
                return psum.tile(list(shape), dtype, tag=f"pair{t}", name=f"{name}{t}")

            wq_sb = persist.tile([P, NCT, H], dt.bfloat16)
            wk_sb = persist.tile([P, NCT, H], dt.bfloat16)
            wv_sb = persist.tile([P, NCT, H], dt.bfloat16)
            nodd_sb = persist.tile([P, 1], dt.float32)
            xo_sb = [
                persist.tile([P, XJ, NQ], dt.bfloat16, name=f"xo{g}")
                for g in range(NG)
            ]
            xs_sb = [
                persist.tile([P, XJ, NQ], dt.bfloat16, name=f"xs{g}")
                for g in range(NG)
            ]
            kT_sb = persist.tile([P, NQ], dt.bfloat16)     # K^T own [h, k]
            vtow = persist.tile([P, NQ], dt.bfloat16)      # V^T own [h, k]
            v_sb = persist.tile([P, NOT, H], dt.bfloat16)  # own V tiles [k, h]
            q_sb = persist.tile([P, T], dt.bfloat16)       # Q^T all [h, q]
            ident = persist.tile([P, P], dt.bfloat16)
            tri = persist.tile([P, P], dt.bfloat16)        # 1 where k <= q
            ones_sb = persist.tile([P, 1], dt.bfloat16)

            # ALL loads on the scalar ring in one strictly-ordered queue so
            # arrival order == consumption order and nothing competes for HBM
            # bandwidth: wk, x-chunk0 (split), wv, x-chunk1, wq, x rest, x-sib
            def x_chunk_ap(xd, g):
                return xd[XJ * P * g:XJ * P * (g + 1), :].rearrange(
                    "(j p) t -> p j t", p=P
                )

            nc.scalar.dma_start(out=wk_sb[:], in_=wk_d[:])
            for jj in range(XJ):  # first chunk: per-c-tile DMA, earliest start
                nc.scalar.dma_start(
                    out=xo_sb[0][:, jj, :],
                    in_=xo_d[P * jj:P * (jj + 1), :],
                )
            nc.scalar.dma_start(out=wv_sb[:], in_=wv_d[:])
            nc.scalar.dma_start(out=xo_sb[1][:], in_=x_chunk_ap(xo_d, 1))
            nc.scalar.dma_start(out=wq_sb[:], in_=wq_d[:])
            nc.sync.dma_start(out=nodd_sb[:], in_=nodd_d[:])
            for g in range(2, NG):
                nc.scalar.dma_start(out=xo_sb[g][:], in_=x_chunk_ap(xo_d, g))
            for g in range(NG):
                nc.scalar.dma_start(out=xs_sb[g][:], in_=x_chunk_ap(xs_d, g))

            make_identity(nc, ident[:])
            make_upper_triangular(nc, tri[:], val=1.0, diag=True)
            nc.vector.memset(ones_sb[:], 1.0)
            # preload the ACT exp table off the attention critical path
            warm_sb = persist.tile([P, 1], dt.float32)
            nc.scalar.activation(
                warm_sb[:], ones_sb[:], mybir.ActivationFunctionType.Exp
            )
            # PE warmup while the first x chunk loads
            warm_ps = pair(0, shape=(P, P), dtype=dt.float32, name="warmps")
            for _ in range(26):
                nc.tensor.matmul(
                    warm_ps[:], lhsT=ident[:], rhs=ident[:],
                    start=True, stop=True,
                )

            # ---- phase 1a: K/V (own keys) + Q-own --------------------------
            # pairs: 0=K 1=V 2=Q-own 3=Q-sib
            ps_k = pair(0, name="psk")
            ps_vt = pair(1, name="psvt")
            ps_qo = pair(2, name="psqo")
            ps_qs = pair(3, name="psqs")

            def proj(ps, w_sb, j, rhs_src, st, sp):
                if WIDE:
                    nc.tensor.matmul(
                        ps[:, 0:NQ], lhsT=w_sb[:, j, :], rhs=rhs_src[:, 0:NQ],
                        start=st, stop=sp,
                    )
                else:
                    for n in range(2):
                        nc.tensor.matmul(
                            ps[:, 512 * n:512 * (n + 1)],
                            lhsT=w_sb[:, j, :],
                            rhs=rhs_src[:, 512 * n:512 * (n + 1)],
                            start=st, stop=sp,
                        )

            for j in range(NCT):
                g, jj = j // XJ, j % XJ
                st, sp = j == 0, j == NCT - 1
                proj(ps_k, wk_sb, j, xo_sb[g][:, jj, :], st, sp)
                proj(ps_vt, wv_sb, j, xo_sb[g][:, jj, :], st, sp)
                proj(ps_qo, wq_sb, j, xo_sb[g][:, jj, :], st, sp)

            # copies: K on ACT, V^T on DVE (parallel), Q-own split
            for n in range(2):
                nc.scalar.copy(
                    out=kT_sb[:, 512 * n:512 * (n + 1)],
                    in_=ps_k[:, 512 * n:512 * (n + 1)],
                )
            for n in range(2):
                nc.vector.tensor_copy(
                    vtow[:, 512 * n:512 * (n + 1)],
                    ps_vt[:, 512 * n:512 * (n + 1)],
                )
            nc.scalar.copy(out=q_sb[:, 0:512], in_=ps_qo[:, 0:512])
            nc.vector.tensor_copy(q_sb[:, 512:1024], ps_qo[:, 512:1024])

            # own V tiles via PE transpose (ping-pong on freed pairs 0/1)
            for m in range(NOT):
                ps_t = pair(m % 2, shape=(P, P), dtype=dt.bfloat16, name="pst")
                nc.tensor.transpose(
                    ps_t[:], vtow[:, m * P:(m + 1) * P], ident[:]
                )
                nc.vector.tensor_copy(v_sb[:, m, :], ps_t[:])

            def qsib_chunk(g):
                for jj in range(XJ):
                    j = XJ * g + jj
                    proj(ps_qs, wq_sb, j, xs_sb[g][:, jj, :],
                         j == 0, j == NCT - 1)

            # ---- attention -------------------------------------------------
            # pass 1 (own-q cols, q_sb[0:1024], tri diag masks):
            #   12 segments of <=512; S slots: p0[0:512], p0[512:], p1[0:512]
            #   halfA (m=0..3, cols [0,512)):   O = p1[:,512:], den = p2[0:1,512:]
            #   halfB (m=0..7, cols [512,1024)): O = p2[:,0:512], den = p2[32:33,512:]
            # pass 2 (sib-q cols, q_sb[1024:2048], nodd masks):
            #   8 full-window segments; S: p0/p1 full; O = p2new; den = p3new[0:1]
            stage = {}

            def emit_s_512(i, segs, s_slots, att_p):
                m, lo, hi, qbase, diag = segs[i]
                n = hi - lo
                ps_s = s_slots[i % 3]
                nc.tensor.matmul(
                    ps_s[:, 0:n],
                    lhsT=kT_sb[:, m * P:(m + 1) * P],
                    rhs=q_sb[:, qbase + lo:qbase + hi],
                    start=True, stop=True, skip_group_check=True,
                )
                a_sb = ephem.tile([P, 512], dt.bfloat16, name="a_sb")
                nc.scalar.activation(
                    a_sb[:, 0:n], ps_s[:, 0:n],
                    mybir.ActivationFunctionType.Exp, scale=SCALE,
                )
                if lo == P * m:
                    if diag:
                        nc.vector.tensor_mul(a_sb[:, 0:P], a_sb[:, 0:P], tri[:])
                    else:
                        nc.vector.tensor_scalar_mul(
                            a_sb[:, 0:P], a_sb[:, 0:P], nodd_sb[:]
                        )
                stage[i] = a_sb

            def run_pass1(p0, p1, p2, hooks):
                segs = []
                for m in range(4):
                    segs.append((m, P * m, 512, 0, True))
                for m in range(NOT):
                    segs.append((m, max(P * m, 512), NQ, 0, True))
                s_slots = [p0[:, 0:512], p0[:, 512:1024], p1[:, 0:512]]

                def od(i):
                    m, lo, hi, qbase, _ = segs[i]
                    n = hi - lo
                    a_sb = stage.pop(i)
                    half = 0 if i < 4 else 1
                    o_ap = p1[:, 512:1024] if half == 0 else p2[:, 0:512]
                    d_ap = (p2[0:1, 512:1024] if half == 0
                            else p2[32:33, 512:1024])
                    off = lo - 512 * half
                    st = m == 0
                    sp = m == (3 if half == 0 else 7)
                    nc.tensor.matmul(
                        o_ap[:, off:off + n], lhsT=v_sb[:, m, :],
                        rhs=a_sb[:, 0:n], start=st, stop=sp,
                        skip_group_check=True,
                    )
                    nc.tensor.matmul(
                        d_ap[:, off:off + n], lhsT=ones_sb[:],
                        rhs=a_sb[:, 0:n], start=st, stop=sp,
                        skip_group_check=True,
                    )
                    if sp:
                        emit_out(0, half, o_ap, d_ap)

                SKEW = 3
                for k in range(12 + SKEW):
                    if k in hooks:
                        hooks[k]()
                    if k < 12:
                        emit_s_512(k, segs, s_slots, None)
                    if k >= SKEW:
                        od(k - SKEW)

            def emit_out(qbase, half, o_ap, d_ap):
                lo = 512 * half
                ot_sb = outp.tile([P, 512], dt.float32, name="ot_sb")
                nc.vector.tensor_copy(ot_sb[:], o_ap[:, 0:512])
                nc.sync.dma_start(
                    out=ot_d[:, qbase + lo:qbase + lo + 512], in_=ot_sb[:]
                )
                den_sb = outp.tile([1, 512], dt.float32, name="den_sb")
                nc.vector.tensor_copy(den_sb[:], d_ap[0:1, 0:512])
                nc.sync.dma_start(
                    out=den_d[:, qbase + lo:qbase + lo + 512], in_=den_sb[:]
                )

            def run_pass2(s_pairs, p_o, p_d):
                def s2(m):
                    c0 = P * m
                    n = NQ - c0
                    ps_s = s_pairs[m % 2]
                    if WIDE or n <= 512:
                        nc.tensor.matmul(
                            ps_s[:, 0:n], lhsT=kT_sb[:, m * P:(m + 1) * P],
                            rhs=q_sb[:, NQ + c0:2 * NQ],
                            start=True, stop=True, skip_group_check=True,
                        )
                    else:
                        # bank-aligned spans: matmul writes must not cross the
                        # 512-col PSUM bank boundary (ACT reads may)
                        for lo, hi in ((c0, 512), (512, NQ)):
                            nc.tensor.matmul(
                                ps_s[:, lo:hi],
                                lhsT=kT_sb[:, m * P:(m + 1) * P],
                                rhs=q_sb[:, NQ + lo:NQ + hi],
                                start=True, stop=True, skip_group_check=True,
                            )
                    a_sb = ephem.tile([P, NQ], dt.bfloat16, name="a2_sb")
                    src = ps_s[:, 0:n] if (WIDE or n <= 512) else ps_s[:, c0:NQ]
                    nc.scalar.activation(
                        a_sb[:, 0:n], src,
                        mybir.ActivationFunctionType.Exp, scale=SCALE,
                    )
                    nc.vector.tensor_scalar_mul(
                        a_sb[:, 0:P], a_sb[:, 0:P], nodd_sb[:]
                    )
                    stage[("p2", m)] = a_sb

                def od2(m):
                    c0 = P * m
                    n = NQ - c0
                    a_sb = stage.pop(("p2", m))
                    st, sp = m == 0, m == NOT - 1
                    spans = ([(c0, NQ)] if WIDE or c0 >= 512
                             else [(c0, 512), (512, NQ)])
                    for lo, hi in spans:
                        nc.tensor.matmul(
                            p_o[:, lo:hi], lhsT=v_sb[:, m, :],
                            rhs=a_sb[:, lo - c0:hi - c0],
                            start=st, stop=sp,
                            skip_group_check=True,
                        )
                        nc.tensor.matmul(
                            p_d[0:1, lo:hi], lhsT=ones_sb[:],
                            rhs=a_sb[:, lo - c0:hi - c0],
                            start=st, stop=sp,
                            skip_group_check=True,
                        )

                SKEW = 2
                for k in range(NOT + SKEW):
                    if k < NOT:
                        s2(k)
                    if k >= SKEW:
                        od2(k - SKEW)
                for half in range(2):
                    emit_out(NQ, half,
                             p_o[:, 512 * half:512 * (half + 1)],
                             p_d[0:1, 512 * half:512 * (half + 1)])

            p0 = pair(0, name="att0")
            p1 = pair(1, name="att1")
            p2 = pair(2, name="att2")
            # pass 1 with Q-sib chunks interleaved (paced by x-sib arrival)
            hooks = {2 * g + 1: (lambda g=g: qsib_chunk(g)) for g in range(6)}
            run_pass1(p0, p1, p2, hooks)
            qsib_chunk(6)
            qsib_chunk(7)
            nc.scalar.copy(out=q_sb[:, NQ:NQ + 512], in_=ps_qs[:, 0:512])
            nc.vector.tensor_copy(q_sb[:, NQ + 512:T], ps_qs[:, 512:1024])
            # pass 2 on fresh pair tiles (0/1 for S, 2 for O, 3 for den)
            p0b = pair(0, name="att0b")
            p1b = pair(1, name="att1b")
            p2b = pair(2, name="att2b")
            p3b = pair(3, name="att3b")
            run_pass2([p0b, p1b], p2b, p3b)

    nc.compile()
    return nc


def _core_cols(par):
    """Permuted x/q column order: own tiles then sibling tiles."""
    own = np.concatenate(
        [np.arange(P * (2 * m + par), P * (2 * m + par) + P) for m in range(NOT)]
    )
    sib = np.concatenate(
        [np.arange(P * (2 * m + 1 - par), P * (2 * m + 1 - par) + P)
         for m in range(NOT)]
    )
    return np.concatenate([own, sib])


def _prep_inputs(x, Wq, Wk, Wv):
    """Build the 8 per-core input maps."""
    def wshape(w):
        return np.ascontiguousarray(
            w.astype(BF16).reshape(NCT, P, H).transpose(1, 0, 2)
        )

    wq_b, wk_b, wv_b = wshape(Wq), wshape(Wk), wshape(Wv)
    x_bf = x.astype(BF16)

    in_maps = []
    for core in range(N_CORES):
        b, par = core // 2, core % 2
        cols = _core_cols(par)
        xT = x_bf[b].T
        nodd = np.full((P, 1), float(1 - par), np.float32)
        in_maps.append({
            "xo": np.ascontiguousarray(xT[:, cols[:NQ]]),
            "xs": np.ascontiguousarray(xT[:, cols[NQ:]]),
            "wq": wq_b, "wk": wk_b, "wv": wv_b,
            "nodd": np.ascontiguousarray(nodd),
        })
    return in_maps


def _assemble(results):
    out = np.empty((B, T, H), np.float32)
    for b in range(B):
        num = np.zeros((H, T), np.float32)
        den = np.zeros((1, T), np.float32)
        for par in range(2):
            r = results[2 * b + par]
            cols = _core_cols(par)
            num[:, cols] += r["ot"]
            den[:, cols] += r["den"]
        out[b] = (num / den).T
    return out


def _run(inputs, trace=False, **spmd_kwargs):
    from concourse.bass_utils import run_bass_kernel_spmd

    if "nc" not in _cache:
        _cache["nc"] = _build()
    nc = _cache["nc"]
    in_maps = _prep_inputs(
        np.asarray(inputs["x"], np.float32),
        np.asarray(inputs["Wq"], np.float32),
        np.asarray(inputs["Wk"], np.float32),
        np.asarray(inputs["Wv"], np.float32),
    )
    res = run_bass_kernel_spmd(
        nc, in_maps, list(range(N_CORES)), trace=trace, **spmd_kwargs
    )
    return _assemble(res.results), res


def kernel(x, Wq, Wk, Wv):
    out, _ = _run({"x": x, "Wq": Wq, "Wk": Wk, "Wv": Wv})
    return out


# revision 13
# speedup vs baseline: 1.0204x; 1.0204x over previous
"""Single-head causal attention (B=4, T=2048, C=2048, H=128) on 8 TRN2 cores.

Sharding: 2 cores per batch (b = core//2, par = core%2), flash-style split
over KEYS. Core (b, par) owns the 8 interleaved key tiles {2m+par}; it
computes K^T/V^T for those 1024 keys only and Q^T for ALL 2048 queries, then
accumulates partial attention sums (O^T = sum_k exp(s) v, den = sum_k exp(s))
over its own keys for every query. The host combines sibling partials:
O = (ot0+ot1)/(den0+den1). Softmax max-shift is skipped (|s| < ~6 here), so
partials combine exactly.

The host permutes x[b]^T columns to [own tiles || sibling tiles], shipped as
two tensors; the own half loads FIRST (all x on one DMA ring, strictly
ordered, so the halves do not compete for HBM bandwidth). K/V/Q-own compute
from the own half; pass-1 attention (own-q columns) overlaps the sibling-half
DMA and the Q-sib projection, which are interleaved into the pass-1 stream.

Causality per own key tile m (global tile 2m+par):
  own-q cols:  window [128m, 1024); first block is the diagonal -> tri mask.
  sib-q cols:  window [128m, 1024)+1024; first block fully allowed (par=0)
    or fully masked (par=1) -> multiply by nodd = 1-par.

PSUM is 4 pair-tags (4KB each); attention carves independent accumulation
regions out of pair tiles by slicing (den vectors live in spare partition
rows, 32-aligned for col_grp). Attention is software-pipelined (S matmuls
skewed ahead of O/den to hide the PE->ACT(exp)->DVE(mask)->PE round-trip);
pass 2 uses full-window segments with one big exp per key tile (ACT
instruction overhead is the attention floor). ~26 dummy warmup matmuls hold
the PE HAM clock warm while the first x chunk loads.
"""

import numpy as np
import ml_dtypes

B, T, C, H = 4, 2048, 2048, 128
P = 128                 # tile edge
NCT = C // P            # 16 contraction chunks
NOT = 8                 # owned key tiles per core
NQ = NOT * P            # 1024 owned key rows per core
N_CORES = 8
SCALE = float(H) ** -0.5
BF16 = ml_dtypes.bfloat16
WIDE = False            # single 1024-col matmuls (ISA bf16 moving max)

_cache = {}


def _build():
    import concourse.bass as bass
    import concourse.mybir as mybir
    import concourse.tile as tile
    from concourse import bacc
    from concourse.masks import make_identity, make_upper_triangular

    dt = mybir.dt
    nc = bacc.Bacc(
        "TRN2",
        target_bir_lowering=False,
        debug=False,
        enable_asserts=False,
        num_devices=N_CORES,
    )

    xo_d = nc.dram_tensor("xo", [C, NQ], dt.bfloat16, kind="ExternalInput").ap()
    xs_d = nc.dram_tensor("xs", [C, NQ], dt.bfloat16, kind="ExternalInput").ap()
    wq_d = nc.dram_tensor("wq", [P, NCT, H], dt.bfloat16, kind="ExternalInput").ap()
    wk_d = nc.dram_tensor("wk", [P, NCT, H], dt.bfloat16, kind="ExternalInput").ap()
    wv_d = nc.dram_tensor("wv", [P, NCT, H], dt.bfloat16, kind="ExternalInput").ap()
    nodd_d = nc.dram_tensor("nodd", [P, 1], dt.float32, kind="ExternalInput").ap()
    ot_d = nc.dram_tensor("ot", [H, T], dt.float32, kind="ExternalOutput").ap()
    den_d = nc.dram_tensor("den", [1, T], dt.float32, kind="ExternalOutput").ap()

    XJ = 2          # c-tiles per x chunk
    NG = NCT // XJ  # 8 chunks per x half

    with tile.TileContext(nc) as tc:
        with (
            tc.tile_pool(name="persist", bufs=1) as persist,
            tc.tile_pool(name="ephem", bufs=6) as ephem,
            tc.tile_pool(name="outp", bufs=2) as outp,
            tc.tile_pool(name="psum", bufs=1, space="PSUM") as psum,
        ):
            def pair(t, shape=(P, 2 * 512), dtype=dt.float32, name="pp"):
                return psum.tile(list(shape), dtype, tag=f"pair{t}", name=f"{name}{t}")

            wq_sb = persist.tile([P, NCT, H], dt.bfloat16)
            wk_sb = persist.tile([P, NCT, H], dt.bfloat16)
            wv_sb = persist.tile([P, NCT, H], dt.bfloat16)
            nodd_sb = persist.tile([P, 1], dt.float32)
            xo_sb = [
                persist.tile([P, XJ, NQ], dt.bfloat16, name=f"xo{g}")
                for g in range(NG)
            ]
            xs_sb = [
                persist.tile([P, XJ, NQ], dt.bfloat16, name=f"xs{g}")
                for g in range(NG)
            ]
            kT_sb = persist.tile([P, NQ], dt.bfloat16)     # K^T own [h, k]
            vtow = persist.tile([P, NQ], dt.bfloat16)      # V^T own [h, k]
            v_sb = persist.tile([P, NOT, H], dt.bfloat16)  # own V tiles [k, h]
            q_sb = persist.tile([P, T], dt.bfloat16)       # Q^T all [h, q]
            ident = persist.tile([P, P], dt.bfloat16)
            tri = persist.tile([P, P], dt.bfloat16)        # 1 where k <= q
            ones_sb = persist.tile([P, 1], dt.bfloat16)

            # ALL loads on the scalar ring in one strictly-ordered queue so
            # arrival order == consumption order and nothing competes for HBM
            # bandwidth: wk, x-chunk0 (split), wv, x-chunk1, wq, x rest, x-sib
            def x_chunk_ap(xd, g):
                return xd[XJ * P * g:XJ * P * (g + 1), :].rearrange(
                    "(j p) t -> p j t", p=P
                )

            nc.scalar.dma_start(out=wk_sb[:], in_=wk_d[:])
            for jj in range(XJ):  # first chunk: per-c-tile DMA, earliest start
                nc.scalar.dma_start(
                    out=xo_sb[0][:, jj, :],
                    in_=xo_d[P * jj:P * (jj + 1), :],
                )
            nc.scalar.dma_start(out=wv_sb[:], in_=wv_d[:])
            nc.scalar.dma_start(out=xo_sb[1][:], in_=x_chunk_ap(xo_d, 1))
            nc.scalar.dma_start(out=wq_sb[:], in_=wq_d[:])
            nc.sync.dma_start(out=nodd_sb[:], in_=nodd_d[:])
            for g in range(2, NG):
                nc.scalar.dma_start(out=xo_sb[g][:], in_=x_chunk_ap(xo_d, g))
            for g in range(NG):
                nc.scalar.dma_start(out=xs_sb[g][:], in_=x_chunk_ap(xs_d, g))

            make_identity(nc, ident[:])
            make_upper_triangular(nc, tri[:], val=1.0, diag=True)
            nc.vector.memset(ones_sb[:], 1.0)
            # preload the ACT exp table off the attention critical path
            warm_sb = persist.tile([P, 1], dt.float32)
            nc.scalar.activation(
                warm_sb[:], ones_sb[:], mybir.ActivationFunctionType.Exp
            )
            # PE warmup while the first x chunk loads
            warm_ps = pair(0, shape=(P, P), dtype=dt.float32, name="warmps")
            for _ in range(26):
                nc.tensor.matmul(
                    warm_ps[:], lhsT=ident[:], rhs=ident[:],
                    start=True, stop=True,
                )

            # ---- phase 1a: K/V (own keys) + Q-own --------------------------
            # pairs: 0=K 1=V 2=Q-own 3=Q-sib
            ps_k = pair(0, name="psk")
            ps_vt = pair(1, name="psvt")
            ps_qo = pair(2, name="psqo")
            ps_qs = pair(3, name="psqs")

            def proj(ps, w_sb, j, rhs_src, st, sp):
                if WIDE:
                    nc.tensor.matmul(
                        ps[:, 0:NQ], lhsT=w_sb[:, j, :], rhs=rhs_src[:, 0:NQ],
                        start=st, stop=sp,
                    )
                else:
                    for n in range(2):
                        nc.tensor.matmul(
                            ps[:, 512 * n:512 * (n + 1)],
                            lhsT=w_sb[:, j, :],
                            rhs=rhs_src[:, 512 * n:512 * (n + 1)],
                            start=st, stop=sp,
                        )

            # chunks 0-1: emission order matches the data arrival order
            # (wk, x0, wv, x1, wq) so the PE FIFO never stalls on late data
            for ps, w_sb, js in (
                (ps_k, wk_sb, (0, 1)), (ps_vt, wv_sb, (0, 1)),
                (ps_k, wk_sb, (2, 3)), (ps_vt, wv_sb, (2, 3)),
                (ps_qo, wq_sb, (0, 1, 2, 3)),
            ):
                for j in js:
                    proj(ps, w_sb, j, xo_sb[j // XJ][:, j % XJ, :],
                         j == 0, False)
            for j in range(4, NCT):
                g, jj = j // XJ, j % XJ
                sp = j == NCT - 1
                proj(ps_k, wk_sb, j, xo_sb[g][:, jj, :], False, sp)
                proj(ps_vt, wv_sb, j, xo_sb[g][:, jj, :], False, sp)
                proj(ps_qo, wq_sb, j, xo_sb[g][:, jj, :], False, sp)

            # copies: K on ACT, V^T on DVE (parallel), Q-own split
            for n in range(2):
                nc.scalar.copy(
                    out=kT_sb[:, 512 * n:512 * (n + 1)],
                    in_=ps_k[:, 512 * n:512 * (n + 1)],
                )
            for n in range(2):
                nc.vector.tensor_copy(
                    vtow[:, 512 * n:512 * (n + 1)],
                    ps_vt[:, 512 * n:512 * (n + 1)],
                )
            nc.scalar.copy(out=q_sb[:, 0:512], in_=ps_qo[:, 0:512])
            nc.vector.tensor_copy(q_sb[:, 512:1024], ps_qo[:, 512:1024])

            # own V tiles via PE transpose (ping-pong on freed pairs 0/1)
            for m in range(NOT):
                ps_t = pair(m % 2, shape=(P, P), dtype=dt.bfloat16, name="pst")
                nc.tensor.transpose(
                    ps_t[:], vtow[:, m * P:(m + 1) * P], ident[:]
                )
                nc.vector.tensor_copy(v_sb[:, m, :], ps_t[:])

            def qsib_chunk(g):
                for jj in range(XJ):
                    j = XJ * g + jj
                    proj(ps_qs, wq_sb, j, xs_sb[g][:, jj, :],
                         j == 0, j == NCT - 1)

            # ---- attention -------------------------------------------------
            # pass 1 (own-q cols, q_sb[0:1024], tri diag masks):
            #   12 segments of <=512; S slots: p0[0:512], p0[512:], p1[0:512]
            #   halfA (m=0..3, cols [0,512)):   O = p1[:,512:], den = p2[0:1,512:]
            #   halfB (m=0..7, cols [512,1024)): O = p2[:,0:512], den = p2[32:33,512:]
            # pass 2 (sib-q cols, q_sb[1024:2048], nodd masks):
            #   8 full-window segments; S: p0/p1 full; O = p2new; den = p3new[0:1]
            stage = {}

            def emit_s_512(i, segs, s_slots, att_p):
                m, lo, hi, qbase, diag = segs[i]
                n = hi - lo
                ps_s = s_slots[i % 3]
                nc.tensor.matmul(
                    ps_s[:, 0:n],
                    lhsT=kT_sb[:, m * P:(m + 1) * P],
                    rhs=q_sb[:, qbase + lo:qbase + hi],
                    start=True, stop=True, skip_group_check=True,
                )
                a_sb = ephem.tile([P, 512], dt.bfloat16, name="a_sb")
                nc.scalar.activation(
                    a_sb[:, 0:n], ps_s[:, 0:n],
                    mybir.ActivationFunctionType.Exp, scale=SCALE,
                )
                if lo == P * m:
                    if diag:
                        nc.vector.tensor_mul(a_sb[:, 0:P], a_sb[:, 0:P], tri[:])
                    else:
                        nc.vector.tensor_scalar_mul(
                            a_sb[:, 0:P], a_sb[:, 0:P], nodd_sb[:]
                        )
                stage[i] = a_sb

            def run_pass1(p0, p1, p2, hooks):
                segs = []
                for m in range(4):
                    segs.append((m, P * m, 512, 0, True))
                for m in range(NOT):
                    segs.append((m, max(P * m, 512), NQ, 0, True))
                s_slots = [p0[:, 0:512], p0[:, 512:1024], p1[:, 0:512]]

                def od(i):
                    m, lo, hi, qbase, _ = segs[i]
                    n = hi - lo
                    a_sb = stage.pop(i)
                    half = 0 if i < 4 else 1
                    o_ap = p1[:, 512:1024] if half == 0 else p2[:, 0:512]
                    d_ap = (p2[0:1, 512:1024] if half == 0
                            else p2[32:33, 512:1024])
                    off = lo - 512 * half
                    st = m == 0
                    sp = m == (3 if half == 0 else 7)
                    nc.tensor.matmul(
                        o_ap[:, off:off + n], lhsT=v_sb[:, m, :],
                        rhs=a_sb[:, 0:n], start=st, stop=sp,
                        skip_group_check=True,
                    )
                    nc.tensor.matmul(
                        d_ap[:, off:off + n], lhsT=ones_sb[:],
                        rhs=a_sb[:, 0:n], start=st, stop=sp,
                        skip_group_check=True,
                    )
                    if sp:
                        emit_out(0, half, o_ap, d_ap)

                SKEW = 3
                for k in range(12 + SKEW):
                    if k in hooks:
                        hooks[k]()
                    if k < 12:
                        emit_s_512(k, segs, s_slots, None)
                    if k >= SKEW:
                        od(k - SKEW)

            def emit_out(qbase, half, o_ap, d_ap):
                lo = 512 * half
                ot_sb = outp.tile([P, 512], dt.float32, name="ot_sb")
                nc.vector.tensor_copy(ot_sb[:], o_ap[:, 0:512])
                nc.sync.dma_start(
                    out=ot_d[:, qbase + lo:qbase + lo + 512], in_=ot_sb[:]
                )
                den_sb = outp.tile([1, 512], dt.float32, name="den_sb")
                nc.vector.tensor_copy(den_sb[:], d_ap[0:1, 0:512])
                nc.sync.dma_start(
                    out=den_d[:, qbase + lo:qbase + lo + 512], in_=den_sb[:]
                )

            def run_pass2(s_pairs, p_o, p_d):
                def s2(m):
                    c0 = P * m
                    n = NQ - c0
                    ps_s = s_pairs[m % 2]
                    if WIDE or n <= 512:
                        nc.tensor.matmul(
                            ps_s[:, 0:n], lhsT=kT_sb[:, m * P:(m + 1) * P],
                            rhs=q_sb[:, NQ + c0:2 * NQ],
                            start=True, stop=True, skip_group_check=True,
                        )
                    else:
                        # bank-aligned spans: matmul writes must not cross the
                        # 512-col PSUM bank boundary (ACT reads may)
                        for lo, hi in ((c0, 512), (512, NQ)):
                            nc.tensor.matmul(
                                ps_s[:, lo:hi],
                                lhsT=kT_sb[:, m * P:(m + 1) * P],
                                rhs=q_sb[:, NQ + lo:NQ + hi],
                                start=True, stop=True, skip_group_check=True,
                            )
                    a_sb = ephem.tile([P, NQ], dt.bfloat16, name="a2_sb")
                    src = ps_s[:, 0:n] if (WIDE or n <= 512) else ps_s[:, c0:NQ]
                    nc.scalar.activation(
                        a_sb[:, 0:n], src,
                        mybir.ActivationFunctionType.Exp, scale=SCALE,
                    )
                    nc.vector.tensor_scalar_mul(
                        a_sb[:, 0:P], a_sb[:, 0:P], nodd_sb[:]
                    )
                    stage[("p2", m)] = a_sb

                def od2(m):
                    c0 = P * m
                    n = NQ - c0
                    a_sb = stage.pop(("p2", m))
                    st, sp = m == 0, m == NOT - 1
                    spans = ([(c0, NQ)] if WIDE or c0 >= 512
                             else [(c0, 512), (512, NQ)])
                    for lo, hi in spans:
                        nc.tensor.matmul(
                            p_o[:, lo:hi], lhsT=v_sb[:, m, :],
                            rhs=a_sb[:, lo - c0:hi - c0],
                            start=st, stop=sp,
                            skip_group_check=True,
                        )
                        nc.tensor.matmul(
                            p_d[0:1, lo:hi], lhsT=ones_sb[:],
                            rhs=a_sb[:, lo - c0:hi - c0],
                            start=st, stop=sp,
                            skip_group_check=True,
                        )

                SKEW = 2
                for k in range(NOT + SKEW):
                    if k < NOT:
                        s2(k)
                    if k >= SKEW:
                        od2(k - SKEW)
                for half in range(2):
                    emit_out(NQ, half,
                             p_o[:, 512 * half:512 * (half + 1)],
                             p_d[0:1, 512 * half:512 * (half + 1)])

            p0 = pair(0, name="att0")
            p1 = pair(1, name="att1")
            p2 = pair(2, name="att2")
            # pass 1 with Q-sib chunks interleaved (paced by x-sib arrival)
            hooks = {2 * g + 1: (lambda g=g: qsib_chunk(g)) for g in range(6)}
            run_pass1(p0, p1, p2, hooks)
            qsib_chunk(6)
            qsib_chunk(7)
            nc.scalar.copy(out=q_sb[:, NQ:NQ + 512], in_=ps_qs[:, 0:512])
            nc.vector.tensor_copy(q_sb[:, NQ + 512:T], ps_qs[:, 512:1024])
            # pass 2 on fresh pair tiles (0/1 for S, 2 for O, 3 for den)
            p0b = pair(0, name="att0b")
            p1b = pair(1, name="att1b")
            p2b = pair(2, name="att2b")
            p3b = pair(3, name="att3b")
            run_pass2([p0b, p1b], p2b, p3b)

    nc.compile()
    return nc


def _core_cols(par):
    """Permuted x/q column order: own tiles then sibling tiles."""
    own = np.concatenate(
        [np.arange(P * (2 * m + par), P * (2 * m + par) + P) for m in range(NOT)]
    )
    sib = np.concatenate(
        [np.arange(P * (2 * m + 1 - par), P * (2 * m + 1 - par) + P)
         for m in range(NOT)]
    )
    return np.concatenate([own, sib])


def _prep_inputs(x, Wq, Wk, Wv):
    """Build the 8 per-core input maps."""
    def wshape(w):
        return np.ascontiguousarray(
            w.astype(BF16).reshape(NCT, P, H).transpose(1, 0, 2)
        )

    wq_b, wk_b, wv_b = wshape(Wq), wshape(Wk), wshape(Wv)
    x_bf = x.astype(BF16)

    in_maps = []
    for core in range(N_CORES):
        b, par = core // 2, core % 2
        cols = _core_cols(par)
        xT = x_bf[b].T
        nodd = np.full((P, 1), float(1 - par), np.float32)
        in_maps.append({
            "xo": np.ascontiguousarray(xT[:, cols[:NQ]]),
            "xs": np.ascontiguousarray(xT[:, cols[NQ:]]),
            "wq": wq_b, "wk": wk_b, "wv": wv_b,
            "nodd": np.ascontiguousarray(nodd),
        })
    return in_maps


def _assemble(results):
    out = np.empty((B, T, H), np.float32)
    for b in range(B):
        num = np.zeros((H, T), np.float32)
        den = np.zeros((1, T), np.float32)
        for par in range(2):
            r = results[2 * b + par]
            cols = _core_cols(par)
            num[:, cols] += r["ot"]
            den[:, cols] += r["den"]
        out[b] = (num / den).T
    return out


def _run(inputs, trace=False, **spmd_kwargs):
    from concourse.bass_utils import run_bass_kernel_spmd

    if "nc" not in _cache:
        _cache["nc"] = _build()
    nc = _cache["nc"]
    in_maps = _prep_inputs(
        np.asarray(inputs["x"], np.float32),
        np.asarray(inputs["Wq"], np.float32),
        np.asarray(inputs["Wk"], np.float32),
        np.asarray(inputs["Wv"], np.float32),
    )
    res = run_bass_kernel_spmd(
        nc, in_maps, list(range(N_CORES)), trace=trace, **spmd_kwargs
    )
    return _assemble(res.results), res


def kernel(x, Wq, Wk, Wv):
    out, _ = _run({"x": x, "Wq": Wq, "Wk": Wk, "Wv": Wv})
    return out
